# revision 26
# baseline (speedup 1.0000x reference)
"""TRN2 Bass/Tile kernel for nn_DotProductAttention (softmax over the QUERY axis).

reference:
    scores  = einsum('bqd,bkd->bqk', q, k) / sqrt(64)
    weights = softmax(scores, axis=1)          # over q, NOT k!
    out     = einsum('bqk,bkd->bqd', weights, v)

Transposed formulation: T = K @ Q^T ([k, q]) puts the softmax axis (q) on the
free axis so the normalizer Z[k] falls out of a free-axis accumulate, and the
1/Z fold goes into V (Vs = V / Z) on the contraction axis of the AV matmul.

Key structure (HW-calibrated):
  - s-permuted staging (s = p*16 + m) so load DMAs move 4KB partition lines.
  - The identity matrix arrives via DMA (a host-provided constant input), so
    the PE transposes are unblocked within ~2us; dummy transpose chains keep
    the PE p-state ramped across the cold start and the B1->B2 boundary.
  - B1 rotates [128,1024] score half-tiles through four 2-bank PSUM slots.
    The exp is split per (batch, chunk) tile between ACT (exact exp, Z via
    accum_out per half) and DVE (Schraudolph fast-exp: int16(scores*A + B)
    written through a bf16-bitcast view of E, Z via one full-tile bf16
    accumulate pass).  ~21:11 ACT:DVE balances the measured engine costs.
  - Z-half combines + reciprocals trail on DVE; the V-row scaling runs on
    the Pool engine (~1.2us/op fixed overhead, but fully parallel).
  - B2 reuses the same PSUM slots: O^T accumulates into two half-width
    accumulators, j-strip outer, so each strip drains, transposes and DMAs
    out while later strips accumulate.

Sharding: B=16 batches, data-parallel over 8 cores => 2 batches per core,
(b, d)-packed into 128-partition tiles so score/AV matmul pairs can run
concurrently in disjoint PE row/column strips.
"""

import math
from contextlib import ExitStack

import numpy as np

import concourse.bass as bass  # noqa: F401
import concourse.mybir as mybir
import concourse.tile as tile
from bass_rust import add_dep_helper
from concourse import bacc, bass_utils

FP32 = mybir.dt.float32
BF16 = mybir.dt.bfloat16
I16 = mybir.dt.int16

N_CORES = 8
B_FULL = 16
BPC = B_FULL // N_CORES  # batches per core = 2
S = 2048
D = 64
NCH = S // 128  # 16 key chunks of 128
HLF = 1024
SCALE = 1.0 / math.sqrt(D)

# Schraudolph fast-exp in bf16 bit-space: bits(e^(s*SCALE)) ~= s*A + B.
# The fp32->int16 convert truncates, so +0.5 folds round-to-nearest into B.
LOG2E = 1.0 / math.log(2.0)
A_FAST = SCALE * LOG2E * 128.0
C_FAST = 5.0
B_FAST = 127.0 * 128.0 - C_FAST + 0.5

# Of the 32 (batch, chunk) tiles, how many get the exact ACT exp.
N_ACT_T = 21
# How many units tile-finalize (combine/recip/V-scale) trails the pipeline.
DEFER = 6
WARMUP_A = 24


def _assignment(n_act, total):
    asg = []
    acc = 0
    for _ in range(total):
        acc += n_act
        if acc >= total:
            acc -= total
            asg.append("A")
        else:
            asg.append("D")
    return asg


def emit_kernel(ctx: ExitStack, tc, q, k, v, o, c_ident, c_zw):
    """Emit the per-core Tile program. q/k/v/o are DRAM APs of [BPC, S, D] f32."""
    nc = tc.nc

    const_pool = ctx.enter_context(tc.tile_pool(name="const", bufs=1))
    big = ctx.enter_context(tc.tile_pool(name="big", bufs=1))

    ident = const_pool.tile([128, 128], FP32, name="ident")
    zw = const_pool.tile([128, 128], BF16, name="zw")

    QT = big.tile([128, S], BF16, name="QT")
    KT = big.tile([128, S], BF16, name="KT")
    qstage = big.tile([128, BPC * NCH * D], FP32, name="qstage")
    kstage = big.tile([128, BPC * NCH * D], FP32, name="kstage")
    V = big.tile([128, BPC * NCH * D], FP32, name="V")
    Vs = big.tile([128, BPC * NCH * D], BF16, name="Vs")
    # per (b, chunk) stats columns: [z_h0, z_h1, z, 1/z]; +8 scratch
    stats = big.tile([128, BPC * NCH * 4 + 8], FP32, name="stats")
    E = big.tile([128, BPC * NCH * S], BF16, name="E")
    OT = big.tile([128, S], FP32, name="OT")
    O_all = big.tile([128, S], FP32, name="O_all")
    zscr = big.tile([128, S], BF16, name="zscr")

    # constants first (tiny), then fat half-DMAs per (tensor, batch)
    nc.sync.dma_start(ident[:], c_ident)
    nc.scalar.dma_start(zw[:], c_zw)
    for src, stg, b, half, eng in (
        (q, qstage, 0, 0, nc.sync),
        (q, qstage, 1, 0, nc.scalar),
        (q, qstage, 0, 1, nc.sync),
        (q, qstage, 1, 1, nc.scalar),
        (k, kstage, 0, 0, nc.sync),
        (k, kstage, 1, 0, nc.scalar),
        (k, kstage, 0, 1, nc.sync),
        (k, kstage, 1, 1, nc.scalar),
    ):
        hw = NCH * D // 2
        eng.dma_start(
            stg[:, b * NCH * D + half * hw : b * NCH * D + (half + 1) * hw],
            src[b].rearrange("(p m) d -> p (m d)", p=128)[:, half * hw : (half + 1) * hw],
        )
    # V rides the SWDGE path (gpsimd) so the two HWDGE queues carry only the
    # critical q/k traffic; V is not needed until well into B1.
    for b in range(BPC):
        nc.gpsimd.dma_start(
            V[:, b * NCH * D : (b + 1) * NCH * D],
            v[b].rearrange("(p m) d -> p (m d)", p=128),
        )

    ps = ctx.enter_context(tc.tile_pool(name="ps", bufs=4, space="PSUM"))

    # ---------------- phase A: PE transposes (f32 in, bf16 out via drain) ---
    # PE p-state pre-warm while the loads are still in flight.  The dummies
    # read a locally-memset junk tile so they depend on no DMA and start
    # within the first microsecond.
    junk = big.tile([128, 128], BF16, name="junk")
    nc.vector.memset(junk[:], 0.0)
    wt = ps.tile([128, 128], FP32, tag="ps", name="warm")
    for _ in range(WARMUP_A):
        nc.tensor.matmul(wt[:], lhsT=junk[:], rhs=junk[:], start=True, stop=True)
    # Four chunks per [128,512] PSUM tile; transpose outputs must start at
    # PSUM partition 0, so the b1 transpose is widened to 128 columns
    # (garbage into rows 0:64) and the b0 transpose overwrites rows 0:64.
    for stg, dst in ((qstage, QT), (kstage, KT)):
        for g in range(4):
            pt4 = ps.tile([128, 512], FP32, tag="ps", name=f"pt4_{dst.name}_{g}")
            for t in range(4):
                m = 4 * g + t
                c1 = NCH * D + m * D - 64
                nc.tensor.transpose(
                    pt4[:, t * 128 : (t + 1) * 128], stg[:, c1 : c1 + 128], ident[:]
                )
                nc.tensor.transpose(
                    pt4[0:64, t * 128 : (t + 1) * 128],
                    stg[:, m * D : (m + 1) * D],
                    ident[:],
                )
            if g % 2 == 0:
                nc.scalar.copy(dst[:, g * 512 : (g + 1) * 512], pt4[:])
            else:
                nc.vector.tensor_copy(dst[:, g * 512 : (g + 1) * 512], pt4[:])

    # ------------- phase B1: scores -> exp (split ACT/DVE per tile) ---------
    tiles = []
    units = []
    for i in range(NCH):
        for h in range(2):
            for b in range(BPC):
                units.append((i, b, h))
        tiles.append((i, 0))
        tiles.append((i, 1))
    # Place the DVE tiles as the b=1 tile of evenly-spread chunks so the
    # unit stream interleaves A/D engines (avoids slot-release stalls from
    # AAAA stretches where ACT alone paces the pipeline).
    n_dve = BPC * NCH - N_ACT_T
    dve_chunks = set()
    acc = 0
    for i in range(NCH):
        acc += n_dve
        if acc >= NCH:
            acc -= NCH
            dve_chunks.add(i)
    while len(dve_chunks) < n_dve:  # n_dve > NCH: double up from the start
        for i in range(NCH):
            if (i, 0) not in dve_chunks and len(dve_chunks) < n_dve:
                dve_chunks.add((i, 0))
    tile_asg = {}
    for i in range(NCH):
        for b in range(BPC):
            is_d = (b == 1 and i in dve_chunks) or ((i, b) in dve_chunks)
            tile_asg[(i, b)] = "D" if is_d else "A"

    def finalize(i, b):
        col = (b * NCH + i) * 4
        vb = (b * NCH + i) * D
        if tile_asg[(i, b)] == "A":
            nc.vector.tensor_add(
                stats[:, col + 2 : col + 3],
                stats[:, col : col + 1],
                stats[:, col + 1 : col + 2],
            )
        nc.vector.reciprocal(stats[:, col + 3 : col + 4], stats[:, col + 2 : col + 3])
        # Early chunks' V-scaling rides the slow-but-parallel Pool engine;
        # late chunks go on DVE so B2's AV sweep is never gated by the Pool
        # queue draining at ~1.2us/op.
        if i < NCH // 2:
            nc.gpsimd.tensor_scalar_mul(
                Vs[:, vb : vb + D], V[:, vb : vb + D], stats[:, col + 3 : col + 4]
            )
        else:
            nc.vector.tensor_scalar_mul(
                Vs[:, vb : vb + D], V[:, vb : vb + D], stats[:, col + 3 : col + 4]
            )

    for u, (i, b, h) in enumerate(units):
        eng = tile_asg[(i, b)]
        sct = ps.tile([128, HLF], FP32, tag="ps", name=f"sc{i}_{b}_{h}")
        for jj in range(2):
            j = 2 * h + jj
            nc.tensor.matmul(
                sct[:, jj * 512 : (jj + 1) * 512],
                lhsT=KT[b * 64 : (b + 1) * 64, i * 128 : (i + 1) * 128],
                rhs=QT[b * 64 : (b + 1) * 64, j * 512 : (j + 1) * 512],
                start=True,
                stop=True,
            )
        col = (b * NCH + i) * 4
        eb = (b * NCH + i) * S + h * HLF
        if eng == "A":
            nc.scalar.activation(
                E[:, eb : eb + HLF],
                sct[:],
                mybir.ActivationFunctionType.Exp,
                scale=SCALE,
                accum_out=stats[:, col + h : col + h + 1],
            )
        else:
            nc.vector.tensor_scalar(
                E[:, eb : eb + HLF].bitcast(I16),
                sct[:],
                A_FAST,
                B_FAST,
                mybir.AluOpType.mult,
                op1=mybir.AluOpType.add,
            )
            if h == 1:
                eb0 = (b * NCH + i) * S
                nc.vector.tensor_scalar(
                    zscr[:],
                    E[:, eb0 : eb0 + S],
                    1.0,
                    None,
                    mybir.AluOpType.mult,
                    op1=mybir.AluOpType.add,
                    accum_out=stats[:, col + 2 : col + 3],
                )
        if u >= DEFER:
            pi, pb, ph_ = units[u - DEFER]
            if ph_ == 1:
                finalize(pi, pb)
    for uu in range(len(units) - DEFER, len(units)):
        pi, pb, ph_ = units[uu]
        if ph_ == 1:
            finalize(pi, pb)

    # ---------------- phase B2: dense AV accumulation, j-strip outer --------
    # Two [128,1024] O^T half-accumulators from the same slot rotation; PE
    # keep-warm dummies bridge the B1->B2 gap before the bank-opening zero
    # matmuls.
    potA = ps.tile([128, HLF], FP32, tag="ps", name="potA")
    potB = ps.tile([128, HLF], FP32, tag="ps", name="potB")
    pots = {0: (potA, 0), 1: (potA, 1), 2: (potB, 0), 3: (potB, 1)}
    zmm = []
    for j in range(4):
        pt_, jj = pots[j]
        zmm.append(
            nc.tensor.matmul(
                pt_[:, jj * 512 : (jj + 1) * 512],
                lhsT=zw[:],
                rhs=QT[:, 0:512],
                start=True,
                stop=False,
                skip_group_check=True,
            )
        )
    o_view = O_all[:].rearrange("p (m b d) -> p m b d", m=NCH, b=BPC, d=D)

    for j in range(4):
        pt_, jj = pots[j]
        for i in range(NCH):
            for b in range(BPC):
                vb = (b * NCH + i) * D
                eb = (b * NCH + i) * S
                mm = nc.tensor.matmul(
                    pt_[b * 64 : (b + 1) * 64, jj * 512 : (jj + 1) * 512],
                    lhsT=Vs[:, vb : vb + D],
                    rhs=E[:, eb + j * 512 : eb + (j + 1) * 512],
                    start=False,
                    stop=(i == NCH - 1),
                    skip_group_check=True,
                )
                if i == 0:
                    add_dep_helper(
                        mm.ins,
                        zmm[j].ins,
                        sync=False,
                        reason="AV accumulation after bank-opening zero matmul",
                    )
        # The last strip's drain chain is exposed kernel tail: pipeline it at
        # half-strip granularity so transposes start after the first half-copy.
        ptc = ps.tile([128, 512], FP32, tag="ps", name=f"ptc_{j}")
        copy_halves = 2 if j == 3 else 1
        cw = 512 // copy_halves
        for hh in range(copy_halves):
            nc.scalar.copy(
                OT[:, j * 512 + hh * cw : j * 512 + (hh + 1) * cw],
                pt_[:, jj * 512 + hh * cw : jj * 512 + (hh + 1) * cw],
            )
            for t in range(hh * (4 // copy_halves), (hh + 1) * (4 // copy_halves)):
                m = 4 * j + t
                nc.tensor.transpose(
                    ptc[:, t * 128 : (t + 1) * 128],
                    OT[:, m * 128 : (m + 1) * 128],
                    ident[:],
                )
        if j % 2 == 0:
            nc.vector.tensor_copy(O_all[:, j * 512 : (j + 1) * 512], ptc[:])
        else:
            nc.scalar.copy(O_all[:, j * 512 : (j + 1) * 512], ptc[:])
        for b in range(BPC):
            eng = nc.sync if b == 0 else nc.scalar
            eng.dma_start(
                o[b].rearrange("(p m) d -> p m d", p=128)[:, 4 * j : 4 * j + 4, :],
                o_view[:, 4 * j : 4 * j + 4, b, :],
            )


_CACHE: dict = {}


def build_program():
    if "nc" in _CACHE:
        return _CACHE["nc"]
    nc = bacc.Bacc("TRN2", target_bir_lowering=False, debug=False)
    q = nc.dram_tensor("q", [BPC, S, D], FP32, kind="ExternalInput").ap()
    k = nc.dram_tensor("k", [BPC, S, D], FP32, kind="ExternalInput").ap()
    v = nc.dram_tensor("v", [BPC, S, D], FP32, kind="ExternalInput").ap()
    ci = nc.dram_tensor("c_ident", [128, 128], FP32, kind="ExternalInput").ap()
    cz = nc.dram_tensor("c_zw", [128, 128], BF16, kind="ExternalInput").ap()
    o = nc.dram_tensor("o", [BPC, S, D], FP32, kind="ExternalOutput").ap()
    with tile.TileContext(nc) as tc:
        with ExitStack() as ctx:
            emit_kernel(ctx, tc, q, k, v, o, ci, cz)
    nc.compile()
    _CACHE["nc"] = nc
    return nc


def make_in_maps(q, k, v):
    q = np.ascontiguousarray(q, dtype=np.float32)
    k = np.ascontiguousarray(k, dtype=np.float32)
    v = np.ascontiguousarray(v, dtype=np.float32)
    assert q.shape == (B_FULL, S, D), q.shape
    import ml_dtypes

    ident = np.eye(128, dtype=np.float32)
    zw = np.zeros((128, 128), dtype=ml_dtypes.bfloat16)
    return [
        {
            "q": np.ascontiguousarray(q[c * BPC : (c + 1) * BPC]),
            "k": np.ascontiguousarray(k[c * BPC : (c + 1) * BPC]),
            "v": np.ascontiguousarray(v[c * BPC : (c + 1) * BPC]),
            "c_ident": ident,
            "c_zw": zw,
        }
        for c in range(N_CORES)
    ]


def kernel(q, k, v, _trace=False):
    nc = build_program()
    in_maps = make_in_maps(q, k, v)
    res = bass_utils.run_bass_kernel_spmd(
        nc, in_maps, core_ids=list(range(N_CORES)), trace=_trace
    )
    out = np.concatenate([r["o"] for r in res.results], axis=0)
    if _trace:
        return out, res
    return out
